# revision 14
# baseline (speedup 1.0000x reference)
"""Additive (Bahdanau) attention kernel for Trainium2, 8 NeuronCores.

Reference computation (B=4, L=1024, D=512, U=64):
    k = x @ Wx                                   [B, L, U]
    q = x @ Wt                                   [B, L, U]
    h = tanh(q[:,i,None,:] + k[:,None,j,:] + bt) [B, L, L, U]
    e = exp(h . Wa + ba)                         [B, L, L]
    a = e / (sum_j e + 1e-7)
    v = a @ x                                    [B, L, D]

Sharding: core c handles batch b=c//2, query half h=c%2 (512 queries), all
1024 keys of that batch. The host hands each core a row-permuted x so the
core's own query rows are always rows 0-511 (key order is softmax-invariant),
letting all 8 cores share one SPMD program.

Algorithm ("Fourier-feature" scores): tanh(t) ~= C0 + sum_m AMPS[m]
sin(OM[m] t) (Gaussian-weighted LSQ fit), which makes the score separable:
S^T = Phik^T @ Phiq with contraction dim 2*64*M — a plain matmul at full PE
rate, eliminating the 33.5M-element tanh of the naive formulation.

Implementation notes (rates measured on HW):
  - x loads as f32 split across sync/gpsimd/scalar HWDGE rings; DVE/ACT
    convert to fp16 purely to feed the DMA xbar transpose (2-byte only,
    14ns per 16x128 tile) — no PE transposes. AV keeps the f32r x.
  - Range reduction for sin (HW table accurate only in [-pi, pi]) in i32
    fixed point using only fast ops (gpsimd tensor ops ~9us and DVE
    tensor_tensor ~6-8us are avoided; tensor_scalar/CAST/ACT ~0.5-1.3us):
      Yi  = i32(round(proj * (OM[m]/2pi*65536) + shift16))   [DVE ts 2-op]
      Fi  = (Yi << 16) >> 16   (centered frac * 65536)       [DVE ts 2-op]
      f   = Sin(Fi * 2pi/65536)                              [ACT, i32 in]
    +0.25-pre-round per-partition shifts make the cos branches; bt folds
    into the k-side shifts. Features are fp16 (score matmuls fp16 x fp16).
    Feature tiles are sectioned [k-half0 | q | k-half1] so score blocks
    g=0..3 only gate on the early sections.
  - Scores per key-block g: psum[128,512] += fcomb[m][:, g]^T @ fqs[m];
    ACT Exp (+ba+C0*sum(Wa)) -> eT f32r; AV (eT^T @ x-block) and den
    accumulate over g in psum. den's 4 regions share one bank, so it uses
    a single accumulation group (per-bank tracking mishandles interleaved
    starts). Tail: reciprocal on DVE, ACT copy-with-scale, out on 3 rings.
"""

import numpy as np
import concourse.bass as bass
import concourse.mybir as mybir
import concourse.tile as tile
from concourse import bacc
from concourse.bass_utils import run_bass_kernel_spmd

F32 = mybir.dt.float32
F32R = mybir.dt.float32r
F16 = mybir.dt.float16
I32 = mybir.dt.int32
Act = mybir.ActivationFunctionType
Alu = mybir.AluOpType

B, L, D, U = 4, 1024, 512, 64
NCORES = 8
NQ = L // 2
NI = NQ // 128  # query blocks (4)
NG = L // 128   # key blocks (8)
DC = D // 128   # D chunks (4)
EPS = 1e-7

# sin-expansion fit of tanh (Gaussian-weighted LSQ, T=11.5, sigma~1.88)
OM = [0.2478004897066865, 0.7442955390848611, 1.248419134511025,
      1.8197788042283967, 2.522505926617766, 3.403049436591139]
AMPS = [1.238725113391988, 0.34002191077806054, 0.1493907690010169,
        0.07190346967903848, 0.02902995166740823, 0.008960461287879442]
C0 = 0.0031388475940618324
M = len(OM)
TWO_PI = float(2 * np.pi)
NC = L + NQ  # combined feature width (1536): [k-h0 512 | q 512 | k-h1 512]

_cached = {}


def _build():
    if "nc" in _cached:
        return _cached["nc"]
    nc = bacc.Bacc("TRN2", target_bir_lowering=False, debug=False,
                   num_devices=NCORES)

    xb = nc.dram_tensor("xb", [L, D], F32R, kind="ExternalInput").ap()
    wxx = nc.dram_tensor("wxx", [128, DC, 128], F16, kind="ExternalInput").ap()
    wtt = nc.dram_tensor("wtt", [128, DC, 128], F16, kind="ExternalInput").ap()
    shk = nc.dram_tensor("shk", [128, M], F32, kind="ExternalInput").ap()
    shq = nc.dram_tensor("shq", [128, M], F32, kind="ExternalInput").ap()
    cvec = nc.dram_tensor("cvec", [128, M], F32, kind="ExternalInput").ap()
    bac = nc.dram_tensor("bac", [128, 1], F32, kind="ExternalInput").ap()
    onesv = nc.dram_tensor("onesv", [128, 2], F32R, kind="ExternalInput").ap()
    vout = nc.dram_tensor("v_out", [NQ, D], F32, kind="ExternalOutput").ap()

    from contextlib import ExitStack

    with tile.TileContext(nc) as tc, ExitStack() as ctx:
        const = ctx.enter_context(tc.tile_pool(name="const", bufs=1))
        xb_sb = [const.tile([128, D], F32R, tag=f"xbg{g}", name=f"xbg{g}")
                 for g in range(NG)]
        xb16 = [const.tile([128, D], F16, tag=f"x16g{g}", name=f"x16g{g}")
                for g in range(NG)]
        # one whole tile per block: the xbar transpose writes a full
        # [128, 4, 128] tile at offset 0 (sliced/offset dests mis-write)
        xtg = [const.tile([128, DC, 128], F16, tag=f"xtg{g}", name=f"xtg{g}")
               for g in range(NG)]
        ktt = const.tile([128, L], F32, tag="ktt", name="ktt")
        qtt = const.tile([128, NQ], F32, tag="qtt", name="qtt")
        fcomb = [const.tile([128, NC], F16, tag=f"fc{m}", name=f"fc{m}")
                 for m in range(M)]
        fqs = [const.tile([128, NQ], F16, tag=f"fqs{m}", name=f"fqs{m}")
               for m in range(M)]
        wxx_sb = const.tile([128, DC, 128], F16, tag="wxx", name="wxx_sb")
        wtt_sb = const.tile([128, DC, 128], F16, tag="wtt", name="wtt_sb")
        shk_sb = const.tile([128, M], F32, tag="shk", name="shk_sb")
        shq_sb = const.tile([128, M], F32, tag="shq", name="shq_sb")
        cvec_sb = const.tile([128, M], F32, tag="cvec", name="cvec_sb")
        bac_sb = const.tile([128, 1], F32, tag="bac", name="bac_sb")
        ones_sb = const.tile([128, 2], F32R, tag="ones", name="ones_sb")
        rcol = const.tile([128, NI, 2], F32, tag="rcol", name="rcol")

        # ------------- input DMAs + fp16 convert + xbar transpose -------
        # The scalar queue carries ONLY the xbar transposes (plus their ACT
        # converts in program order): mixing plain HBM loads onto the same
        # queue as dma_start_transpose corrupts the transposes.
        xb_r = xb.rearrange("(g p) d -> p g d", p=128)
        for g in range(NG):
            ring = nc.sync if g % 2 == 0 else nc.gpsimd
            ring.dma_start(out=xb_sb[g][:], in_=xb_r[:, g, :])
        nc.sync.dma_start(out=wxx_sb[:], in_=wxx[:])
        nc.sync.dma_start(out=wtt_sb[:], in_=wtt[:])
        nc.sync.dma_start(out=shk_sb[:], in_=shk[:])
        nc.sync.dma_start(out=shq_sb[:], in_=shq[:])
        nc.sync.dma_start(out=cvec_sb[:], in_=cvec[:])
        nc.sync.dma_start(out=bac_sb[:], in_=bac[:])
        nc.sync.dma_start(out=ones_sb[:], in_=onesv[:])
        for g in range(NG):
            nc.scalar.activation(xb16[g][:], xb_sb[g][:], Act.Copy)
            nc.scalar.dma_start_transpose(out=xtg[g][:], in_=xb16[g][:])

        red = ctx.enter_context(tc.tile_pool(name="red", bufs=2))
        et_pool = ctx.enter_context(tc.tile_pool(name="et", bufs=3))
        vo_pool = ctx.enter_context(tc.tile_pool(name="vo", bufs=2))
        scq = ctx.enter_context(
            tc.tile_pool(name="scq", bufs=3, space="PSUM"))
        v_pool = ctx.enter_context(
            tc.tile_pool(name="vps", bufs=1, space="PSUM"))
        v_ps = [v_pool.tile([128, D], F32, tag=f"v{i}", name=f"v{i}")
                for i in range(NI)]
        den_ps = v_pool.tile([128, NI, 2], F32, tag="den", name="den_ps")

        # ---------------- projections ----------------
        qp = scq.tile([128, NQ], F32, tag="big", name="qp")
        for g in range(4):
            for c in range(DC):
                nc.tensor.matmul(qp[:, g * 128:(g + 1) * 128],
                                 wtt_sb[:, c, :], xtg[g][:, c, :],
                                 start=(c == 0), stop=(c == DC - 1))
        nc.vector.tensor_copy(qtt[:], qp[:])
        for H in range(2):
            hs = slice(H * 512, (H + 1) * 512)
            kp = scq.tile([128, 512], F32, tag="big", name=f"kp{H}")
            for gg in range(4):
                g = H * 4 + gg
                for c in range(DC):
                    nc.tensor.matmul(kp[:, gg * 128:(gg + 1) * 128],
                                     wxx_sb[:, c, :], xtg[g][:, c, :],
                                     start=(c == 0), stop=(c == DC - 1))
            nc.vector.tensor_copy(ktt[:, hs], kp[:])

        # ---------------- features ----------------
        # fcomb layout: [k 0:1024 | q 1024:1536]
        for m in range(M):
            w16 = float(OM[m] / TWO_PI * 65536.0)
            yi = red.tile([128, NC], I32, tag="yi", name="yi", bufs=3)
            nc.vector.tensor_scalar(yi[:, 0:L], ktt[:], w16,
                                    shk_sb[:, m:m + 1], Alu.mult, Alu.add)
            nc.vector.tensor_scalar(yi[:, L:NC], qtt[:], w16,
                                    shq_sb[:, m:m + 1], Alu.mult, Alu.add)
            fi = red.tile([128, NC], I32, tag="fi", name="fi", bufs=3)
            nc.vector.tensor_scalar(fi[:], yi[:], 16, 16,
                                    Alu.logical_shift_left,
                                    Alu.arith_shift_right)
            nc.scalar.activation(fcomb[m][:], fi[:], Act.Sin,
                                 scale=float(TWO_PI / 65536.0))
            nc.vector.tensor_scalar_mul(fqs[m][:], fcomb[m][:, L:NC],
                                        cvec_sb[:, m:m + 1])

        # ---------------- main loop: scores -> exp -> AV ----------------
        def ksec(g):
            return slice(g * 128, (g + 1) * 128)

        def sc_block(g):
            sc = scq.tile([128, NQ], F32, tag="big", name=f"sc{g}")
            gs = ksec(g)
            for m in range(M):
                nc.tensor.matmul(sc[:], fcomb[m][:, gs], fqs[m][:],
                                 start=(m == 0), stop=(m == M - 1))
            et_t = et_pool.tile([128, NQ], F32R, tag="et", name=f"et{g}")
            nc.scalar.activation(et_t[:], sc[:], Act.Exp, bias=bac_sb[:])
            return et_t

        def av_block(g, et_t):
            for ib in range(NI):
                ibs = slice(ib * 128, (ib + 1) * 128)
                nc.tensor.matmul(v_ps[ib][:], et_t[:, ibs], xb_sb[g][:],
                                 start=(g == 0), stop=(g == NG - 1))
                nc.tensor.matmul(den_ps[:, ib, :], et_t[:, ibs], ones_sb[:],
                                 start=(g == 0 and ib == 0),
                                 stop=(g == NG - 1 and ib == NI - 1))

        ets = {}
        ets[0] = sc_block(0)
        ets[1] = sc_block(1)
        for g in range(2, NG):
            av_block(g - 2, ets.pop(g - 2))
            ets[g] = sc_block(g)
        av_block(NG - 2, ets.pop(NG - 2))
        av_block(NG - 1, ets.pop(NG - 1))

        # ---------------- normalize + out ----------------
        nc.vector.tensor_scalar_add(rcol[:], den_ps[:], float(EPS))
        nc.vector.reciprocal(rcol[:], rcol[:])
        for ib in range(NI):
            v_sb = vo_pool.tile([128, D], F32, tag="vo", name="v_sb")
            nc.scalar.activation(v_sb[:], v_ps[ib][:], Act.Copy,
                                 scale=rcol[:, ib, 0:1])
            ring = [nc.sync, nc.gpsimd, nc.scalar, nc.sync][ib]
            ring.dma_start(out=vout[ib * 128:(ib + 1) * 128, :], in_=v_sb[:])

    nc.compile()
    _cached["nc"] = nc
    return nc


def _host_prep(x, Wx, Wt, bt, Wa, ba):
    x = np.ascontiguousarray(x, dtype=np.float32)
    Wx = np.ascontiguousarray(Wx, dtype=np.float32)
    Wt = np.ascontiguousarray(Wt, dtype=np.float32)
    bt = np.asarray(bt, dtype=np.float32).reshape(U)
    Wa = np.asarray(Wa, dtype=np.float32).reshape(U)
    ba = np.asarray(ba, dtype=np.float32).reshape(1)

    # doubled-column projection stationaries (fp16): out row p carries u=p%64
    wxx = np.empty((128, DC, 128), dtype=np.float16)
    wtt = np.empty((128, DC, 128), dtype=np.float16)
    for c in range(DC):
        blkx = Wx[c * 128:(c + 1) * 128, :]
        blkt = Wt[c * 128:(c + 1) * 128, :]
        wxx[:, c, :] = np.concatenate([blkx, blkx], axis=1).astype(np.float16)
        wtt[:, c, :] = np.concatenate([blkt, blkt], axis=1).astype(np.float16)

    pmod = np.arange(128) % 64
    top = (np.arange(128) < 64).astype(np.float32)
    shk = np.empty((128, M), dtype=np.float32)
    shq = np.empty((128, M), dtype=np.float32)
    cvec = np.empty((128, M), dtype=np.float32)
    for m in range(M):
        # k side stacked [cos; sin]; q side stacked [sin; cos]
        shk[:, m] = (top * 0.25 + bt[pmod] * OM[m] / TWO_PI) * 65536.0
        shq[:, m] = (1.0 - top) * 0.25 * 65536.0
        cvec[:, m] = AMPS[m] * Wa[pmod]
    bac = np.full((128, 1), ba[0] + C0 * Wa.sum(), dtype=np.float32)
    onesv = np.ones((128, 2), dtype=np.float32)

    shared = {"wxx": wxx, "wtt": wtt, "shk": shk, "shq": shq,
              "cvec": cvec, "bac": bac, "onesv": onesv}
    in_maps = []
    for c in range(NCORES):
        b, h = c // 2, c % 2
        if h == 0:
            xbp = x[b]
        else:
            xbp = np.concatenate([x[b, NQ:], x[b, :NQ]], axis=0)
        mm = dict(shared)
        mm["xb"] = np.ascontiguousarray(xbp)
        in_maps.append(mm)
    return in_maps


def kernel(x, Wx, Wt, bt, Wa, ba):
    nc = _build()
    in_maps = _host_prep(x, Wx, Wt, bt, Wa, ba)
    res = run_bass_kernel_spmd(nc, in_maps, core_ids=list(range(NCORES)))
    out = np.empty((B, L, D), dtype=np.float32)
    for c in range(NCORES):
        b, h = c // 2, c % 2
        out[b, h * NQ:(h + 1) * NQ, :] = res.results[c]["v_out"]
    return out


if __name__ == "__main__":
    rng = np.random.default_rng(0)
    x = rng.standard_normal((B, L, D), dtype=np.float32)
    Wx = (rng.standard_normal((D, U), dtype=np.float32) * 0.06).astype(np.float32)
    Wt = (rng.standard_normal((D, U), dtype=np.float32) * 0.06).astype(np.float32)
    bt = np.zeros(U, dtype=np.float32)
    Wa = (rng.standard_normal((U, 1), dtype=np.float32) * 0.17).astype(np.float32)
    ba = np.zeros(1, dtype=np.float32)
    v = kernel(x=x, Wx=Wx, Wt=Wt, bt=bt, Wa=Wa, ba=ba)
    print("kernel ran, out shape", v.shape)
